# revision 45
# baseline (speedup 1.0000x reference)
"""GCN layer (message passing) on 8 Trainium2 NeuronCores.

out = relu(((D^-1/2 A D^-1/2) X) @ W.T) + X

Strategy (dst-sharded graph partitioning, bf16 device path):
  - Destination nodes sharded across 8 cores (12500 nodes each); every core
    gathers from the full pre-normalized feature table h = X * D^-1/2 stored
    bf16 (256B rows); the host concatenates the 8 output slices.
  - The SWDGE gather descriptor generation on the GpSimd Q7 pair is the
    bottleneck resource (~2.1ns/descriptor, and the ucode pads every call to
    full 128-slot chunks). To minimize descriptors, gathers are merged
    across quads of 4 dst tiles: one call per (quad, src bucket) -> 100
    calls instead of 392, and the chunk-rounding waste is paid once per
    quad instead of once per tile. Chunks on a tile boundary feed both
    tiles' accumulations (the one-hot matrices zero out foreign slots).
  - Both degree norms leave the device inner loop: the src-side norm is
    folded into h (host), the dst-side norm is applied as a per-partition
    scale fused into the final ReLU.
  - Device, per tile: the one-hot scatter matrix S[e, (j,d)] = (ld[e,j]==d)
    over the tile's chunk window is built by ONE DVE tensor_tensor
    (is_equal) against a constant iota-d plane, with ld broadcast along d
    via a stride-0 AP. W_t bf16 matmuls accumulate zT[f,d] in PSUM;
    zT -> bf16 SBUF (DVE copy); y = zT.T @ W.T (PE); ReLU with per-partition
    scale norm[dst] (ACT); residual add (DVE); outputs staged in groups of
    7 tiles and stored with one DMA per group.
  - Partial tail chunks of each gather are pre-zeroed on the ~idle ACT
    engine so 0 * stale-NaN cannot poison the PSUM accumulation; all other
    pad slots gather row 0 of the bucket and are killed by ld=-1.
"""

import math

import ml_dtypes
import numpy as np

import concourse.bacc as bacc
import concourse.mybir as mybir
from concourse.bass_utils import run_bass_kernel_spmd
from concourse.tile import TileContext

P = 128
N_CORES = 8
BUCKET_MAX = 25000  # int16 gather indices: bucket the node space
G_IO = 5  # tiles per residual-load/output-store group
G_Q = 4  # tiles per merged-gather quad
G_IDX = 4  # quads per staged idx-table load


def _prepare(features, W, edge_src, edge_dst, n_cores=N_CORES, bucket_max=BUCKET_MAX):
    """Partition the graph by dst core / gather quad / src bucket."""
    features = np.asarray(features, dtype=np.float32)
    W = np.asarray(W, dtype=np.float32)
    edge_src = np.asarray(edge_src, dtype=np.int32)
    edge_dst = np.asarray(edge_dst, dtype=np.int32)

    n_nodes, d = features.shape
    assert d == P
    assert n_nodes % n_cores == 0
    npc = n_nodes // n_cores
    n_tiles = math.ceil(npc / P)
    rows_last = npc - (n_tiles - 1) * P
    nb = math.ceil(n_nodes / bucket_max)
    B = math.ceil(n_nodes / nb)
    assert B <= 32768

    degs = np.bincount(edge_dst, minlength=n_nodes).astype(np.float32)
    norm = 1.0 / np.sqrt(np.maximum(degs, 1.0), dtype=np.float32)
    h16 = (features * norm[:, None]).astype(ml_dtypes.bfloat16)

    core_of = edge_dst // npc

    # per-core sorted edge lists and per-(tile,bucket) counts
    per_core = []
    counts_all = np.zeros((n_cores, n_tiles, nb), np.int64)
    for k in range(n_cores):
        sel = np.flatnonzero(core_of == k)
        src_k = edge_src[sel]
        ldst = edge_dst[sel] - k * npc
        tile_of = ldst // P
        bucket = src_k // B
        order = np.lexsort((src_k, bucket, tile_of))
        sel = sel[order]
        gid = tile_of[order] * nb + bucket[order]
        counts = np.bincount(gid, minlength=n_tiles * nb).reshape(n_tiles, nb)
        counts_all[k] = counts
        per_core.append((sel, gid, (ldst[order] % P).astype(np.float32)))

    # static per-(tile,bucket) gather segment sizes: max across cores
    n_tb = counts_all.max(axis=0)  # [n_tiles, nb]

    # ---- static quad-merged gather layout ----
    nq = math.ceil(n_tiles / G_Q)
    quads = [(q * G_Q, min((q + 1) * G_Q, n_tiles)) for q in range(nq)]
    off_tb = np.zeros((n_tiles, nb), np.int64)  # slot offset in (q,b) stream
    L_qb = np.zeros((nq, nb), np.int64)  # stream length per (quad, bucket)
    CB_qb = np.zeros((nq, nb), np.int64)  # chunks per (quad, bucket)
    gc0_qb = np.zeros((nq, nb), np.int64)  # chunk base within quad X
    icol_qb = np.zeros((nq, nb), np.int64)  # idx col base (global)
    CQ_q = np.zeros(nq, np.int64)
    icol = 0
    for q, (t0, t1) in enumerate(quads):
        gc = 0
        for b in range(nb):
            off = 0
            for t in range(t0, t1):
                off_tb[t, b] = off
                off += n_tb[t, b]
            L_qb[q, b] = off
            CB_qb[q, b] = (off + P - 1) // P
            gc0_qb[q, b] = gc
            gc += CB_qb[q, b]
            icol_qb[q, b] = icol
            icol += (off + 15) // 16
        CQ_q[q] = gc
    total_icols = int(icol)
    CQmax = int(CQ_q.max())

    # idx table is staged in groups of G_IDX quads (keeping it fully
    # resident would crowd out the third X buffer)
    n_ig = math.ceil(nq / G_IDX)
    ig_col0 = np.zeros(n_ig + 1, np.int64)
    for g in range(n_ig):
        q_end = min((g + 1) * G_IDX, nq)
        ig_col0[g + 1] = (
            int(icol_qb[q_end, 0]) if q_end < nq else total_icols
        )
    ICGmax = int(np.diff(ig_col0).max()) if n_ig else 0

    # per-tile chunk windows (within the quad X buffer) and ld columns
    chunk_map = []  # [t] -> list of quad-chunk indices
    wj0_tb = np.zeros((n_tiles, nb), np.int64)  # window start in chunk_map[t]
    for t in range(n_tiles):
        q = t // G_Q
        cm = []
        for b in range(nb):
            wj0_tb[t, b] = len(cm)
            n = n_tb[t, b]
            if n == 0:
                continue
            o = off_tb[t, b]
            c_lo = o // P
            c_hi = (o + n + P - 1) // P
            cm.extend(int(gc0_qb[q, b]) + c for c in range(c_lo, c_hi))
        chunk_map.append(cm)
    W_t = np.array([len(cm) for cm in chunk_map], np.int64)
    Wmax = int(W_t.max())
    L0_t = np.concatenate([[0], np.cumsum(W_t)])[:-1]
    total_L = int(W_t.sum())

    layout = dict(
        n_nodes=n_nodes,
        npc=npc,
        n_tiles=n_tiles,
        rows_last=rows_last,
        nb=nb,
        B=B,
        quads=quads,
        L_qb=L_qb,
        CB_qb=CB_qb,
        gc0_qb=gc0_qb,
        icol_qb=icol_qb,
        CQmax=CQmax,
        ig_col0=ig_col0,
        ICGmax=ICGmax,
        chunk_map=chunk_map,
        W_t=W_t,
        Wmax=Wmax,
        L0_t=L0_t,
        total_L=total_L,
        total_icols=total_icols,
    )

    # constant iota-d plane: col (j, d) -> d
    iota_d = np.tile(
        np.arange(P, dtype=np.float32).astype(ml_dtypes.bfloat16), (P, Wmax)
    )
    wt16 = np.ascontiguousarray(W.T).astype(ml_dtypes.bfloat16)  # wt[f, o] = W[o, f]

    in_maps = []
    for k in range(n_cores):
        sel, gid, ld_sorted = per_core[k]
        group_start = np.zeros(n_tiles * nb, np.int64)
        cnts = counts_all[k].reshape(-1)
        group_start[1:] = np.cumsum(cnts)[:-1]
        rank = np.arange(len(sel)) - group_start[gid]
        t_of = gid // nb
        b_of = gid % nb
        q_of = t_of // G_Q

        # stream position of each edge within its (quad, bucket) stream
        pos = off_tb[t_of, b_of] + rank

        # idx array [16, total_icols] then replicated to 128 partitions
        idx16 = np.zeros((16, total_icols), np.int16)
        ic = icol_qb[q_of, b_of] + pos // 16
        idx16[pos % 16, ic] = (edge_src[sel] - b_of * B).astype(np.int16)
        idxm = np.tile(idx16, (8, 1))

        # ld array [128, total_L] bf16: local dst per (tile window col, slot)
        ld = np.full((P, total_L), -1.0, np.float32)
        j = wj0_tb[t_of, b_of] + (pos // P - off_tb[t_of, b_of] // P)
        ld[pos % P, L0_t[t_of] + j] = ld_sorted
        ld16 = ld.astype(ml_dtypes.bfloat16)

        # per-partition dst norm per tile (1.0 on the unused tail rows)
        normT = np.ones((P, n_tiles), np.float32)
        ncol = norm[k * npc : (k + 1) * npc]
        nfull = (n_tiles - 1) * P
        normT[:, : n_tiles - 1] = ncol[:nfull].reshape(n_tiles - 1, P).T
        normT[:rows_last, n_tiles - 1] = ncol[nfull:]

        in_maps.append(
            {
                "feats": h16,
                "idxm": np.ascontiguousarray(idxm),
                "ld": np.ascontiguousarray(ld16),
                "wt": wt16,
                "iotad": iota_d,
                "normt": normT,
                "resid": np.ascontiguousarray(features[k * npc : (k + 1) * npc]),
            }
        )
    return in_maps, layout


def _build_program(layout):
    f32 = mybir.dt.float32
    bf16 = mybir.dt.bfloat16
    i16 = mybir.dt.int16
    n_nodes = layout["n_nodes"]
    npc = layout["npc"]
    n_tiles = layout["n_tiles"]
    rows_last = layout["rows_last"]
    nb = layout["nb"]
    B = layout["B"]
    quads = layout["quads"]
    L_qb = layout["L_qb"]
    CB_qb = layout["CB_qb"]
    gc0_qb = layout["gc0_qb"]
    icol_qb = layout["icol_qb"]
    CQmax = layout["CQmax"]
    ig_col0 = layout["ig_col0"]
    ICGmax = layout["ICGmax"]
    chunk_map = layout["chunk_map"]
    W_t = layout["W_t"]
    Wmax = layout["Wmax"]
    L0_t = layout["L0_t"]
    total_L = layout["total_L"]
    total_icols = layout["total_icols"]

    nc = bacc.Bacc(num_swdge_queues=4)
    feats = nc.declare_dram_parameter("feats", [n_nodes, P], bf16, isOutput=False)
    idxm = nc.declare_dram_parameter("idxm", [P, total_icols], i16, isOutput=False)
    ldp = nc.declare_dram_parameter("ld", [P, total_L], bf16, isOutput=False)
    wt = nc.declare_dram_parameter("wt", [P, P], bf16, isOutput=False)
    iotad = nc.declare_dram_parameter("iotad", [P, Wmax * P], bf16, isOutput=False)
    normt = nc.declare_dram_parameter("normt", [P, n_tiles], f32, isOutput=False)
    resid = nc.declare_dram_parameter("resid", [npc, P], f32, isOutput=False)
    out = nc.declare_dram_parameter("out", [npc, P], f32, isOutput=True)

    n_groups = math.ceil(n_tiles / G_IO)
    with TileContext(nc) as tc:
        with (
            tc.tile_pool(name="const", bufs=1) as constp,
            tc.tile_pool(name="idx", bufs=3) as idxp,
            tc.tile_pool(name="x", bufs=3) as xp,
            tc.tile_pool(name="s", bufs=3) as sp,
            tc.tile_pool(name="zps", bufs=6, space="PSUM") as zpsp,
            tc.tile_pool(name="yps", bufs=2, space="PSUM") as ypsp,
            tc.tile_pool(name="zt", bufs=3) as ztp,
            tc.tile_pool(name="y", bufs=3) as yp,
            tc.tile_pool(name="res", bufs=2) as resp,
            tc.tile_pool(name="og", bufs=2) as ogp,
        ):
            # consts go on the scalar HWDGE ring so the sync ring serves
            # the first idx-group load (and the first gathers) immediately
            ld_sb = constp.tile([P, total_L], bf16)
            nc.scalar.dma_start(out=ld_sb[:], in_=ldp[:, :])
            iota_sb = constp.tile([P, Wmax * P], bf16)
            nc.scalar.dma_start(out=iota_sb[:], in_=iotad[:, :])
            wt_sb = constp.tile([P, P], bf16)
            nc.scalar.dma_start(out=wt_sb[:], in_=wt[:, :])
            norm_sb = constp.tile([P, n_tiles], f32)
            nc.scalar.dma_start(out=norm_sb[:], in_=normt[:, :])

            res_g = None
            og = None
            idx_sb = None
            ic0 = 0
            for q, (t0, t1) in enumerate(quads):
                if q % G_IDX == 0:
                    g = q // G_IDX
                    ic0 = int(ig_col0[g])
                    ic1 = int(ig_col0[g + 1])
                    idx_sb = idxp.tile([P, max(ICGmax, 1)], i16, tag="idx")
                    nc.sync.dma_start(
                        out=idx_sb[:, : ic1 - ic0], in_=idxm[:, ic0:ic1]
                    )
                X = xp.tile([P, CQmax * P], bf16, tag="X")
                for b in range(nb):
                    L = int(L_qb[q, b])
                    if L == 0:
                        continue
                    cb = int(CB_qb[q, b])
                    g0 = int(gc0_qb[q, b])
                    io = int(icol_qb[q, b]) - ic0
                    icb = (L + 15) // 16
                    if L % P:
                        # the gather leaves partitions >= L%128 of its last
                        # chunk unwritten; pre-zero that chunk (on the ~idle
                        # ACT engine) so 0 * stale-NaN can't poison the
                        # one-hot matmul
                        nc.scalar.memzero(X[:, (g0 + cb - 1) * P : (g0 + cb) * P])
                    nc.gpsimd.dma_gather(
                        out_ap=X[:, g0 * P : (g0 + cb) * P].rearrange(
                            "p (c e) -> p c e", e=P
                        ),
                        in_ap=feats[b * B : min((b + 1) * B, n_nodes), :],
                        idxs_ap=idx_sb[:, io : io + icb],
                        num_idxs=L,
                        num_idxs_reg=L,
                        elem_size=P,
                        single_packet=False,
                        queue_num=b % 4,
                    )

                for t in range(t0, t1):
                    g = t // G_IO
                    j = t - g * G_IO
                    if j == 0:
                        gt = min(G_IO, n_tiles - g * G_IO)
                        full_t = (
                            gt if g < n_groups - 1 or rows_last == P else gt - 1
                        )
                        res_g = resp.tile([P, G_IO * P], f32, tag="res")
                        og = ogp.tile([P, G_IO * P], f32, tag="og")
                        r0 = g * G_IO * P
                        if full_t:
                            nc.sync.dma_start(
                                out=res_g[:, : full_t * P].rearrange(
                                    "p (t f) -> p t f", f=P
                                ),
                                in_=resid[r0 : r0 + full_t * P, :].rearrange(
                                    "(t p) f -> p t f", p=P
                                ),
                            )
                        if full_t < gt:
                            nc.sync.dma_start(
                                out=res_g[:rows_last, full_t * P : (full_t + 1) * P],
                                in_=resid[r0 + full_t * P : npc, :],
                            )

                    Wt = int(W_t[t])
                    L0 = int(L0_t[t])

                    # one-hot scatter matrix for the whole tile in ONE DVE
                    # op: S[e, (j,d)] = (ld[e,j] == d), ld broadcast along d
                    # via stride-0 AP
                    S_full = sp.tile([P, Wmax * P], bf16, tag="S")
                    S = S_full[:, : Wt * P]
                    ld_b = (
                        ld_sb[:, L0 : L0 + Wt]
                        .rearrange("p (c u) -> p c u", u=1)
                        .broadcast_to([P, Wt, P])
                    )
                    nc.vector.tensor_tensor(
                        out=S.rearrange("p (c e) -> p c e", e=P),
                        in0=ld_b,
                        in1=iota_sb[:, : Wt * P].rearrange("p (c e) -> p c e", e=P),
                        op=mybir.AluOpType.is_equal,
                    )

                    # zT[f, d] += X_c[e, f].T @ S_j[e, d]
                    z_ps = zpsp.tile([P, P], f32)
                    for wj, gc in enumerate(chunk_map[t]):
                        nc.tensor.matmul(
                            out=z_ps[:],
                            lhsT=X[:, gc * P : (gc + 1) * P],
                            rhs=S[:, wj * P : (wj + 1) * P],
                            start=(wj == 0),
                            stop=(wj == Wt - 1),
                        )

                    zT_sb = ztp.tile([P, P], bf16, tag="zT")
                    nc.vector.tensor_copy(out=zT_sb[:], in_=z_ps[:])
                    # y[d, o] = zT[f, d].T @ wt[f, o]
                    y_ps = ypsp.tile([P, P], f32)
                    nc.tensor.matmul(
                        out=y_ps[:], lhsT=zT_sb[:], rhs=wt_sb[:], start=True, stop=True
                    )

                    rows = P if t < n_tiles - 1 else rows_last
                    # fused ReLU(y * norm[dst]) with per-partition scale
                    y_sb = yp.tile([P, P], f32, tag="y")
                    nc.scalar.activation(
                        out=y_sb[:],
                        in_=y_ps[:],
                        func=mybir.ActivationFunctionType.Relu,
                        scale=norm_sb[:, t : t + 1],
                    )
                    nc.vector.tensor_add(
                        out=og[:rows, j * P : (j + 1) * P],
                        in0=y_sb[:rows],
                        in1=res_g[:rows, j * P : (j + 1) * P],
                    )

                    if j == G_IO - 1 or t == n_tiles - 1:
                        gt = j + 1
                        full_t = (
                            gt if t < n_tiles - 1 or rows_last == P else gt - 1
                        )
                        r0 = g * G_IO * P
                        if full_t:
                            nc.sync.dma_start(
                                out=out[r0 : r0 + full_t * P, :].rearrange(
                                    "(t p) f -> p t f", p=P
                                ),
                                in_=og[:, : full_t * P].rearrange(
                                    "p (t f) -> p t f", f=P
                                ),
                            )
                        if full_t < gt:
                            nc.sync.dma_start(
                                out=out[r0 + full_t * P : npc, :],
                                in_=og[:rows_last, full_t * P : (full_t + 1) * P],
                            )
    nc.finalize()
    return nc


def _run(features, W, edge_src, edge_dst, trace=False, **spmd_kwargs):
    in_maps, layout = _prepare(features, W, edge_src, edge_dst)
    nc = _build_program(layout)
    br = run_bass_kernel_spmd(
        nc, in_maps, core_ids=list(range(N_CORES)), trace=trace, **spmd_kwargs
    )
    outs = [r["out"] for r in br.results]
    full = np.concatenate(outs, axis=0).astype(np.float32)
    return full, br


def kernel(features, W, edge_src, edge_dst):
    out, _ = _run(features, W, edge_src, edge_dst, trace=False)
    return out


# revision 46
# speedup vs baseline: 1.2500x; 1.2500x over previous
"""GCN layer (message passing) on 8 Trainium2 NeuronCores.

out = relu(((D^-1/2 A D^-1/2) X) @ W.T) + X

Strategy (dst-sharded graph partitioning, bf16 device path):
  - Destination nodes sharded across 8 cores (12500 nodes each); every core
    gathers from the full pre-normalized feature table h = X * D^-1/2 stored
    bf16 (256B rows); the host concatenates the 8 output slices.
  - The SWDGE gather descriptor generation on the GpSimd Q7 pair is the
    bottleneck resource (~2.1ns/descriptor, and the ucode pads every call to
    full 128-slot chunks). To minimize descriptors, gathers are merged
    across quads of 4 dst tiles: one call per (quad, src bucket) -> 100
    calls instead of 392, and the chunk-rounding waste is paid once per
    quad instead of once per tile. Chunks on a tile boundary feed both
    tiles' accumulations (the one-hot matrices zero out foreign slots).
  - Both degree norms leave the device inner loop: the src-side norm is
    folded into h (host), the dst-side norm is applied as a per-partition
    scale fused into the final ReLU.
  - Device, per tile: the one-hot scatter matrix S[e, (j,d)] = (ld[e,j]==d)
    over the tile's chunk window is built by ONE DVE tensor_tensor
    (is_equal) against a constant iota-d plane, with ld broadcast along d
    via a stride-0 AP. W_t bf16 matmuls accumulate zT[f,d] in PSUM;
    zT -> bf16 SBUF (DVE copy); y = zT.T @ W.T (PE); ReLU with per-partition
    scale norm[dst] (ACT); residual add (DVE); outputs staged in groups of
    7 tiles and stored with one DMA per group.
  - Partial tail chunks of each gather are pre-zeroed on the ~idle ACT
    engine so 0 * stale-NaN cannot poison the PSUM accumulation; all other
    pad slots gather row 0 of the bucket and are killed by ld=-1.
"""

import math

import ml_dtypes
import numpy as np

import concourse.bacc as bacc
import concourse.mybir as mybir
from concourse.bass_utils import run_bass_kernel_spmd
from concourse.tile import TileContext

P = 128
N_CORES = 8
BUCKET_MAX = 25000  # int16 gather indices: bucket the node space
G_IO = 5  # tiles per residual-load/output-store group
G_Q = 4  # tiles per merged-gather quad
G_IDX = 4  # quads per staged idx-table load


def _prepare(features, W, edge_src, edge_dst, n_cores=N_CORES, bucket_max=BUCKET_MAX):
    """Partition the graph by dst core / gather quad / src bucket."""
    features = np.asarray(features, dtype=np.float32)
    W = np.asarray(W, dtype=np.float32)
    edge_src = np.asarray(edge_src, dtype=np.int32)
    edge_dst = np.asarray(edge_dst, dtype=np.int32)

    n_nodes, d = features.shape
    assert d == P
    assert n_nodes % n_cores == 0
    npc = n_nodes // n_cores
    n_tiles = math.ceil(npc / P)
    rows_last = npc - (n_tiles - 1) * P
    nb = math.ceil(n_nodes / bucket_max)
    B = math.ceil(n_nodes / nb)
    assert B <= 32768

    degs = np.bincount(edge_dst, minlength=n_nodes).astype(np.float32)
    norm = 1.0 / np.sqrt(np.maximum(degs, 1.0), dtype=np.float32)
    h16 = (features * norm[:, None]).astype(ml_dtypes.bfloat16)

    core_of = edge_dst // npc

    # per-core sorted edge lists and per-(tile,bucket) counts
    per_core = []
    counts_all = np.zeros((n_cores, n_tiles, nb), np.int64)
    for k in range(n_cores):
        sel = np.flatnonzero(core_of == k)
        src_k = edge_src[sel]
        ldst = edge_dst[sel] - k * npc
        tile_of = ldst // P
        bucket = src_k // B
        order = np.lexsort((src_k, bucket, tile_of))
        sel = sel[order]
        gid = tile_of[order] * nb + bucket[order]
        counts = np.bincount(gid, minlength=n_tiles * nb).reshape(n_tiles, nb)
        counts_all[k] = counts
        per_core.append((sel, gid, (ldst[order] % P).astype(np.float32)))

    # static per-(tile,bucket) gather segment sizes: max across cores
    n_tb = counts_all.max(axis=0)  # [n_tiles, nb]

    # ---- static quad-merged gather layout ----
    nq = math.ceil(n_tiles / G_Q)
    quads = [(q * G_Q, min((q + 1) * G_Q, n_tiles)) for q in range(nq)]
    off_tb = np.zeros((n_tiles, nb), np.int64)  # slot offset in (q,b) stream
    L_qb = np.zeros((nq, nb), np.int64)  # stream length per (quad, bucket)
    CB_qb = np.zeros((nq, nb), np.int64)  # chunks per (quad, bucket)
    gc0_qb = np.zeros((nq, nb), np.int64)  # chunk base within quad X
    icol_qb = np.zeros((nq, nb), np.int64)  # idx col base (global)
    CQ_q = np.zeros(nq, np.int64)
    icol = 0
    for q, (t0, t1) in enumerate(quads):
        gc = 0
        for b in range(nb):
            off = 0
            for t in range(t0, t1):
                off_tb[t, b] = off
                off += n_tb[t, b]
            L_qb[q, b] = off
            CB_qb[q, b] = (off + P - 1) // P
            gc0_qb[q, b] = gc
            gc += CB_qb[q, b]
            icol_qb[q, b] = icol
            icol += (off + 15) // 16
        CQ_q[q] = gc
    total_icols = int(icol)
    CQmax = int(CQ_q.max())

    # idx table is staged in groups of G_IDX quads (keeping it fully
    # resident would crowd out the third X buffer)
    n_ig = math.ceil(nq / G_IDX)
    ig_col0 = np.zeros(n_ig + 1, np.int64)
    for g in range(n_ig):
        q_end = min((g + 1) * G_IDX, nq)
        ig_col0[g + 1] = (
            int(icol_qb[q_end, 0]) if q_end < nq else total_icols
        )
    ICGmax = int(np.diff(ig_col0).max()) if n_ig else 0

    # per-tile chunk windows (within the quad X buffer) and ld columns
    chunk_map = []  # [t] -> list of quad-chunk indices
    wj0_tb = np.zeros((n_tiles, nb), np.int64)  # window start in chunk_map[t]
    for t in range(n_tiles):
        q = t // G_Q
        cm = []
        for b in range(nb):
            wj0_tb[t, b] = len(cm)
            n = n_tb[t, b]
            if n == 0:
                continue
            o = off_tb[t, b]
            c_lo = o // P
            c_hi = (o + n + P - 1) // P
            cm.extend(int(gc0_qb[q, b]) + c for c in range(c_lo, c_hi))
        chunk_map.append(cm)
    W_t = np.array([len(cm) for cm in chunk_map], np.int64)
    Wmax = int(W_t.max())
    L0_t = np.concatenate([[0], np.cumsum(W_t)])[:-1]
    total_L = int(W_t.sum())

    layout = dict(
        n_nodes=n_nodes,
        npc=npc,
        n_tiles=n_tiles,
        rows_last=rows_last,
        nb=nb,
        B=B,
        quads=quads,
        L_qb=L_qb,
        CB_qb=CB_qb,
        gc0_qb=gc0_qb,
        icol_qb=icol_qb,
        CQmax=CQmax,
        ig_col0=ig_col0,
        ICGmax=ICGmax,
        chunk_map=chunk_map,
        W_t=W_t,
        Wmax=Wmax,
        L0_t=L0_t,
        total_L=total_L,
        total_icols=total_icols,
    )

    # constant iota-d plane: col (j, d) -> d
    iota_d = np.tile(
        np.arange(P, dtype=np.float32).astype(ml_dtypes.bfloat16), (P, Wmax)
    )
    wt16 = np.ascontiguousarray(W.T).astype(ml_dtypes.bfloat16)  # wt[f, o] = W[o, f]

    in_maps = []
    for k in range(n_cores):
        sel, gid, ld_sorted = per_core[k]
        group_start = np.zeros(n_tiles * nb, np.int64)
        cnts = counts_all[k].reshape(-1)
        group_start[1:] = np.cumsum(cnts)[:-1]
        rank = np.arange(len(sel)) - group_start[gid]
        t_of = gid // nb
        b_of = gid % nb
        q_of = t_of // G_Q

        # stream position of each edge within its (quad, bucket) stream
        pos = off_tb[t_of, b_of] + rank

        # idx array [16, total_icols] then replicated to 128 partitions
        idx16 = np.zeros((16, total_icols), np.int16)
        ic = icol_qb[q_of, b_of] + pos // 16
        idx16[pos % 16, ic] = (edge_src[sel] - b_of * B).astype(np.int16)
        idxm = np.tile(idx16, (8, 1))

        # ld array [128, total_L] bf16: local dst per (tile window col, slot)
        ld = np.full((P, total_L), -1.0, np.float32)
        j = wj0_tb[t_of, b_of] + (pos // P - off_tb[t_of, b_of] // P)
        ld[pos % P, L0_t[t_of] + j] = ld_sorted
        ld16 = ld.astype(ml_dtypes.bfloat16)

        # per-partition dst norm per tile (1.0 on the unused tail rows)
        normT = np.ones((P, n_tiles), np.float32)
        ncol = norm[k * npc : (k + 1) * npc]
        nfull = (n_tiles - 1) * P
        normT[:, : n_tiles - 1] = ncol[:nfull].reshape(n_tiles - 1, P).T
        normT[:rows_last, n_tiles - 1] = ncol[nfull:]

        in_maps.append(
            {
                "feats": h16,
                "idxm": np.ascontiguousarray(idxm),
                "ld": np.ascontiguousarray(ld16),
                "wt": wt16,
                "iotad": iota_d,
                "normt": normT,
                "resid": np.ascontiguousarray(features[k * npc : (k + 1) * npc]),
            }
        )
    return in_maps, layout


def _build_program(layout):
    f32 = mybir.dt.float32
    bf16 = mybir.dt.bfloat16
    i16 = mybir.dt.int16
    n_nodes = layout["n_nodes"]
    npc = layout["npc"]
    n_tiles = layout["n_tiles"]
    rows_last = layout["rows_last"]
    nb = layout["nb"]
    B = layout["B"]
    quads = layout["quads"]
    L_qb = layout["L_qb"]
    CB_qb = layout["CB_qb"]
    gc0_qb = layout["gc0_qb"]
    icol_qb = layout["icol_qb"]
    CQmax = layout["CQmax"]
    ig_col0 = layout["ig_col0"]
    ICGmax = layout["ICGmax"]
    chunk_map = layout["chunk_map"]
    W_t = layout["W_t"]
    Wmax = layout["Wmax"]
    L0_t = layout["L0_t"]
    total_L = layout["total_L"]
    total_icols = layout["total_icols"]

    nc = bacc.Bacc(num_swdge_queues=4)
    feats = nc.declare_dram_parameter("feats", [n_nodes, P], bf16, isOutput=False)
    idxm = nc.declare_dram_parameter("idxm", [P, total_icols], i16, isOutput=False)
    ldp = nc.declare_dram_parameter("ld", [P, total_L], bf16, isOutput=False)
    wt = nc.declare_dram_parameter("wt", [P, P], bf16, isOutput=False)
    iotad = nc.declare_dram_parameter("iotad", [P, Wmax * P], bf16, isOutput=False)
    normt = nc.declare_dram_parameter("normt", [P, n_tiles], f32, isOutput=False)
    resid = nc.declare_dram_parameter("resid", [npc, P], f32, isOutput=False)
    out = nc.declare_dram_parameter("out", [npc, P], f32, isOutput=True)

    n_groups = math.ceil(n_tiles / G_IO)
    with TileContext(nc) as tc:
        with (
            tc.tile_pool(name="const", bufs=1) as constp,
            tc.tile_pool(name="idx", bufs=3) as idxp,
            tc.tile_pool(name="x", bufs=3) as xp,
            tc.tile_pool(name="s", bufs=3) as sp,
            tc.tile_pool(name="zps", bufs=4, space="PSUM") as zpsp,
            tc.tile_pool(name="yps", bufs=4, space="PSUM") as ypsp,
            tc.tile_pool(name="zt", bufs=3) as ztp,
            tc.tile_pool(name="y", bufs=3) as yp,
            tc.tile_pool(name="res", bufs=2) as resp,
            tc.tile_pool(name="og", bufs=2) as ogp,
        ):
            # consts go on the scalar HWDGE ring so the sync ring serves
            # the first idx-group load (and the first gathers) immediately
            ld_sb = constp.tile([P, total_L], bf16)
            nc.scalar.dma_start(out=ld_sb[:], in_=ldp[:, :])
            iota_sb = constp.tile([P, Wmax * P], bf16)
            nc.scalar.dma_start(out=iota_sb[:], in_=iotad[:, :])
            wt_sb = constp.tile([P, P], bf16)
            nc.scalar.dma_start(out=wt_sb[:], in_=wt[:, :])
            norm_sb = constp.tile([P, n_tiles], f32)
            nc.scalar.dma_start(out=norm_sb[:], in_=normt[:, :])

            res_g = None
            og = None
            idx_sb = None
            ic0 = 0
            for q, (t0, t1) in enumerate(quads):
                if q % G_IDX == 0:
                    g = q // G_IDX
                    ic0 = int(ig_col0[g])
                    ic1 = int(ig_col0[g + 1])
                    idx_sb = idxp.tile([P, max(ICGmax, 1)], i16, tag="idx")
                    nc.sync.dma_start(
                        out=idx_sb[:, : ic1 - ic0], in_=idxm[:, ic0:ic1]
                    )
                X = xp.tile([P, CQmax * P], bf16, tag="X")
                for b in range(nb):
                    L = int(L_qb[q, b])
                    if L == 0:
                        continue
                    cb = int(CB_qb[q, b])
                    g0 = int(gc0_qb[q, b])
                    io = int(icol_qb[q, b]) - ic0
                    icb = (L + 15) // 16
                    if L % P:
                        # the gather leaves partitions >= L%128 of its last
                        # chunk unwritten; pre-zero that chunk (on the ~idle
                        # ACT engine) so 0 * stale-NaN can't poison the
                        # one-hot matmul
                        nc.scalar.memzero(X[:, (g0 + cb - 1) * P : (g0 + cb) * P])
                    nc.gpsimd.dma_gather(
                        out_ap=X[:, g0 * P : (g0 + cb) * P].rearrange(
                            "p (c e) -> p c e", e=P
                        ),
                        in_ap=feats[b * B : min((b + 1) * B, n_nodes), :],
                        idxs_ap=idx_sb[:, io : io + icb],
                        num_idxs=L,
                        num_idxs_reg=L,
                        elem_size=P,
                        single_packet=False,
                        queue_num=b % 4,
                    )

                for t in range(t0, t1):
                    g = t // G_IO
                    j = t - g * G_IO
                    if j == 0:
                        gt = min(G_IO, n_tiles - g * G_IO)
                        full_t = (
                            gt if g < n_groups - 1 or rows_last == P else gt - 1
                        )
                        res_g = resp.tile([P, G_IO * P], f32, tag="res")
                        og = ogp.tile([P, G_IO * P], f32, tag="og")
                        r0 = g * G_IO * P
                        if full_t:
                            nc.sync.dma_start(
                                out=res_g[:, : full_t * P].rearrange(
                                    "p (t f) -> p t f", f=P
                                ),
                                in_=resid[r0 : r0 + full_t * P, :].rearrange(
                                    "(t p) f -> p t f", p=P
                                ),
                            )
                        if full_t < gt:
                            nc.sync.dma_start(
                                out=res_g[:rows_last, full_t * P : (full_t + 1) * P],
                                in_=resid[r0 + full_t * P : npc, :],
                            )

                    Wt = int(W_t[t])
                    L0 = int(L0_t[t])

                    # one-hot scatter matrix for the whole tile in ONE DVE
                    # op: S[e, (j,d)] = (ld[e,j] == d), ld broadcast along d
                    # via stride-0 AP
                    S_full = sp.tile([P, Wmax * P], bf16, tag="S")
                    S = S_full[:, : Wt * P]
                    ld_b = (
                        ld_sb[:, L0 : L0 + Wt]
                        .rearrange("p (c u) -> p c u", u=1)
                        .broadcast_to([P, Wt, P])
                    )
                    nc.vector.tensor_tensor(
                        out=S.rearrange("p (c e) -> p c e", e=P),
                        in0=ld_b,
                        in1=iota_sb[:, : Wt * P].rearrange("p (c e) -> p c e", e=P),
                        op=mybir.AluOpType.is_equal,
                    )

                    # zT[f, d] += X_c[e, f].T @ S_j[e, d]
                    z_ps = zpsp.tile([P, P], f32)
                    for wj, gc in enumerate(chunk_map[t]):
                        nc.tensor.matmul(
                            out=z_ps[:],
                            lhsT=X[:, gc * P : (gc + 1) * P],
                            rhs=S[:, wj * P : (wj + 1) * P],
                            start=(wj == 0),
                            stop=(wj == Wt - 1),
                        )

                    zT_sb = ztp.tile([P, P], bf16, tag="zT")
                    nc.vector.tensor_copy(out=zT_sb[:], in_=z_ps[:])
                    # y[d, o] = zT[f, d].T @ wt[f, o]
                    y_ps = ypsp.tile([P, P], f32)
                    nc.tensor.matmul(
                        out=y_ps[:], lhsT=zT_sb[:], rhs=wt_sb[:], start=True, stop=True
                    )

                    rows = P if t < n_tiles - 1 else rows_last
                    # fused ReLU(y * norm[dst]) with per-partition scale
                    y_sb = yp.tile([P, P], f32, tag="y")
                    nc.scalar.activation(
                        out=y_sb[:],
                        in_=y_ps[:],
                        func=mybir.ActivationFunctionType.Relu,
                        scale=norm_sb[:, t : t + 1],
                    )
                    nc.vector.tensor_add(
                        out=og[:rows, j * P : (j + 1) * P],
                        in0=y_sb[:rows],
                        in1=res_g[:rows, j * P : (j + 1) * P],
                    )

                    if j == G_IO - 1 or t == n_tiles - 1:
                        gt = j + 1
                        full_t = (
                            gt if t < n_tiles - 1 or rows_last == P else gt - 1
                        )
                        r0 = g * G_IO * P
                        if full_t:
                            nc.sync.dma_start(
                                out=out[r0 : r0 + full_t * P, :].rearrange(
                                    "(t p) f -> p t f", p=P
                                ),
                                in_=og[:, : full_t * P].rearrange(
                                    "p (t f) -> p t f", f=P
                                ),
                            )
                        if full_t < gt:
                            nc.sync.dma_start(
                                out=out[r0 + full_t * P : npc, :],
                                in_=og[:rows_last, full_t * P : (full_t + 1) * P],
                            )
    nc.finalize()
    return nc


def _run(features, W, edge_src, edge_dst, trace=False, **spmd_kwargs):
    in_maps, layout = _prepare(features, W, edge_src, edge_dst)
    nc = _build_program(layout)
    br = run_bass_kernel_spmd(
        nc, in_maps, core_ids=list(range(N_CORES)), trace=trace, **spmd_kwargs
    )
    outs = [r["out"] for r in br.results]
    full = np.concatenate(outs, axis=0).astype(np.float32)
    return full, br


def kernel(features, W, edge_src, edge_dst):
    out, _ = _run(features, W, edge_src, edge_dst, trace=False)
    return out
